# revision 1
# baseline (speedup 1.0000x reference)
"""Trainium2 Bass kernel for nn_ExpertFFN (MoE routing, E=8 experts, top-2).

Math (matching the reference exactly):
  xt = x.reshape(N, H); logits = xt @ Wr + br; gates = softmax(logits)
  G, idx = top2(gates); P = one_hot(idx)           # (N,2), (N,2)
  tok[e,k,:]  = sum_n P[n,k,e] * xt[n,:]           # (E,2,H)
  mid[e] = gelu(tok[e] @ W1[e] + b1[e])            # exact erf-gelu
  Eo[e]  = mid[e] @ W2[e] + b2[e]                  # (E,2,H)
  y[n]   = sum_k G[n,k] * (sum_e Eo[e,k,:])        # combine over ALL experts

Sharding across 8 NeuronCores:
  - router + dispatch: token-parallel (1024 tokens/core), partial tok
    summed across cores with an in-kernel ReduceScatter -> core c holds
    tok rows [2c, 2c+2) = expert c's two aggregate tokens.
  - FFN: expert-parallel; core c streams W1[c], W2[c] (128 MiB) once.
  - combine: AllReduce of the tiny (2, H) expert outputs, then each core
    computes y for its own 1024 tokens.
The kernel is HBM-bound on the 1 GiB weight stream (memory regime).
"""

import numpy as np

import concourse.bass as bass
import concourse.bacc as bacc
import concourse.mybir as mybir
import concourse.tile as tile
from concourse.masks import make_identity
from concourse.bass_utils import run_bass_kernel_spmd

F32 = mybir.dt.float32
AX = mybir.AxisListType.X
ALU = mybir.AluOpType
ACT_GELU = mybir.ActivationFunctionType.Gelu
ACT_EXP = mybir.ActivationFunctionType.Exp

# problem dims
B, S, H, F, E, TOPK = 4, 2048, 2048, 8192, 8, 2
N = B * S
NCORES = 8
P = 128


def build_expert_ffn(nc_cores=NCORES, h=H, f=F, e=E, tn=None, w_bufs=3,
                     x_bufs=2, collectives=True, ffn_reps=1, kernel_reps=1,
                     phase_stop=5):
    """Trace + compile the per-core SPMD program. All cores run the same
    program; per-core behavior comes from per-core input data."""
    tn = tn if tn is not None else N // nc_cores  # tokens per core
    nt = tn // P        # token tiles
    hc = h // P         # h chunks of 128
    fb = 512            # f-group width
    ng = f // fb        # f groups
    ek = e * TOPK       # rows of tok
    hb = h // 512       # output column banks

    nc = bacc.Bacc("TRN2", target_bir_lowering=False, debug=False,
                   num_devices=nc_cores)

    xs = nc.dram_tensor("xs", [tn, h], F32, kind="ExternalInput")
    Wr = nc.dram_tensor("Wr", [h, e], F32, kind="ExternalInput")
    brb = nc.dram_tensor("brb", [1, e], F32, kind="ExternalInput")
    W1c = nc.dram_tensor("W1c", [h, f], F32, kind="ExternalInput")
    b1c = nc.dram_tensor("b1c", [1, f], F32, kind="ExternalInput")
    W2c = nc.dram_tensor("W2c", [f, h], F32, kind="ExternalInput")
    b2c = nc.dram_tensor("b2c", [1, h], F32, kind="ExternalInput")
    yo = nc.dram_tensor("y", [tn, h], F32, kind="ExternalOutput")

    groups = [list(range(nc_cores))]

    with tile.TileContext(nc) as tc:
        with (
            tc.tile_pool(name="const", bufs=1) as cpool,
            tc.tile_pool(name="sb", bufs=1) as sb,
            tc.tile_pool(name="xpool", bufs=x_bufs) as xpool,
            tc.tile_pool(name="xtpool", bufs=4) as xtpool,
            tc.tile_pool(name="small", bufs=2) as small,
            tc.tile_pool(name="wpool", bufs=w_bufs) as wpool,
            tc.tile_pool(name="ypool", bufs=2) as ypool,
            tc.tile_pool(name="dram", bufs=1, space="DRAM") as dram,
        ):
            # ---- constants ----
            ident = cpool.tile([P, P], F32)
            make_identity(nc, ident[:])
            ones1 = cpool.tile([1, P], F32)
            nc.gpsimd.memset(ones1[:], 1.0)
            br1 = cpool.tile([1, e], F32)
            nc.sync.dma_start(br1[:], brb[:])
            wr_sb = cpool.tile([P, hc, e], F32)
            for j in range(hc):
                nc.sync.dma_start(wr_sb[:, j, :], Wr[j * P:(j + 1) * P, :])
            b2_sb = cpool.tile([2, h], F32)
            for r in range(2):
                nc.sync.dma_start(b2_sb[r:r + 1, :], b2c[:])
            # b1 in col-tiled layout: rows 32q+r hold b1[q*(f/4) : (q+1)*(f/4)]
            fq = f // 4
            b1t = cpool.tile([P, fq], F32)
            b1v = b1c.ap().rearrange("o (q i) -> o q i", q=4)
            for q in range(4):
                for r in range(2):
                    nc.sync.dma_start(b1t[32 * q + r:32 * q + r + 1, :],
                                      b1v[:, q, :])

            def trace_once(krep):
                # persistent router outputs
                gv_all = sb.tile([P, nt, 2], F32)      # top-2 gate values per token
                tokp_sb = sb.tile([ek, h], F32)        # partial tok (this core's tokens)

                # prefetch the first W1 slabs NOW — these DMAs have no deps and
                # sit first on the ACT sequencer, so the weight stream starts at
                # t=0 and overlaps the whole router phase
                w1v = W1c.ap().rearrange("(j p) f -> p j f", p=P)
                w_pre = []
                if ffn_reps == 1 and krep == 0:
                    for j in range(w_bufs):
                        w1s = wpool.tile([P, f], F32, tag="w", name=f"w1pre{j}")
                        nc.scalar.dma_start(w1s[:], w1v[:, j, :])
                        w_pre.append(w1s)

                # ================= phase R: router + dispatch =================
                with (
                    tc.tile_pool(name="ps_r", bufs=1, space="PSUM") as ps_r,
                    tc.tile_pool(name="ps_tok", bufs=1, space="PSUM") as ps_tok,
                ):
                    # broadcast br across 128 partitions with a K=1 outer product
                    br_ps = ps_r.tile([P, e], F32, tag="lg", bufs=2)
                    nc.tensor.matmul(br_ps[:], ones1[:], br1[:], start=True, stop=True)
                    br_bc = cpool.tile([P, e], F32)
                    nc.vector.tensor_copy(br_bc[:], br_ps[:])

                    # dispatch psum, col-tiled: h-bank b lives at partitions
                    # [32b, 32b+ek) of one (128, 512) bank
                    tokp = ps_tok.tile([P, 512], F32)

                    for t in range(nt):
                        x_t = xpool.tile([P, h], F32, tag="x")
                        nc.sync.dma_start(x_t[:], xs[t * P:(t + 1) * P, :])

                        # logits (128 tok, e) accumulated over h chunks
                        lg = ps_r.tile([P, e], F32, tag="lg", bufs=2)
                        for j in range(hc):
                            xt_ps = ps_r.tile([P, P], F32, tag="xtps", bufs=4)
                            nc.tensor.transpose(
                                xt_ps[:], x_t[:, j * P:(j + 1) * P], ident[:])
                            xt_sb = xtpool.tile([P, P], F32, tag="xt")
                            nc.vector.tensor_copy(xt_sb[:], xt_ps[:])
                            nc.tensor.matmul(lg[:], xt_sb[:], wr_sb[:, j, :],
                                             start=(j == 0), stop=(j == hc - 1))

                        # softmax over e (free axis) + top-2 + one-hot dispatch mask
                        lgb = small.tile([P, e], F32, tag="lgb")
                        nc.vector.tensor_add(lgb[:], lg[:], br_bc[:])
                        nmax = small.tile([P, 1], F32, tag="nmax")
                        nc.vector.reduce_max(nmax[:], lgb[:], axis=AX, negate=True)
                        ex = small.tile([P, e], F32, tag="ex")
                        nc.scalar.activation(ex[:], lgb[:], ACT_EXP, bias=nmax[:])
                        ssum = small.tile([P, 1], F32, tag="ssum")
                        nc.vector.reduce_sum(ssum[:], ex[:], axis=AX)
                        rinv = small.tile([P, 1], F32, tag="rinv")
                        nc.vector.reciprocal(rinv[:], ssum[:])
                        gates = small.tile([P, e], F32, tag="gates")
                        nc.vector.tensor_scalar_mul(gates[:], ex[:], rinv[:])

                        msel = small.tile([P, e, 2], F32, tag="msel")
                        m1 = gv_all[:, t, 0:1]
                        nc.vector.reduce_max(m1, gates[:], axis=AX)
                        nc.vector.tensor_scalar(msel[:, :, 0], gates[:], m1, None,
                                                op0=ALU.is_equal)
                        g2 = small.tile([P, e], F32, tag="g2")
                        nc.vector.tensor_scalar(g2[:], msel[:, :, 0], -2.0, None,
                                                op0=ALU.mult)
                        nc.vector.tensor_add(g2[:], g2[:], gates[:])
                        m2 = gv_all[:, t, 1:2]
                        nc.vector.reduce_max(m2, g2[:], axis=AX)
                        nc.vector.tensor_scalar(msel[:, :, 1], g2[:], m2, None,
                                                op0=ALU.is_equal)

                        # dispatch: tokp += msel.T @ x_t (4 h-banks col-tiled)
                        msel2 = msel[:].rearrange("p e k -> p (e k)")
                        for b in range(hb):
                            nc.tensor.matmul(
                                tokp[32 * b:32 * b + ek, :], msel2,
                                x_t[:, b * 512:(b + 1) * 512],
                                start=(t == 0), stop=(t == nt - 1),
                                tile_position=(0, 32 * b))

                    for b in range(hb):
                        nc.vector.tensor_copy(tokp_sb[:, b * 512:(b + 1) * 512],
                                              tokp[32 * b:32 * b + ek, :])

                if phase_stop < 2:
                    return
                # ============ ReduceScatter: sum tok over cores, keep own expert ====
                cc1_in = dram.tile([ek, h], F32)
                cc1_out = dram.tile([TOPK, h], F32)
                nc.sync.dma_start(cc1_in[:], tokp_sb[:])
                if collectives:
                    nc.gpsimd.collective_compute(
                        "ReduceScatter", ALU.add, replica_groups=groups,
                        ins=[cc1_in.opt()], outs=[cc1_out.opt()])
                else:
                    nc.sync.dma_start(cc1_out[:], cc1_in[0:TOPK, :])
                tokc = sb.tile([TOPK, h], F32)
                nc.sync.dma_start(tokc[:], cc1_out[:])

                # tokT: (h, 2) laid out as hc chunks of (128, 2)
                tokT = sb.tile([P, hc, 2], F32)
                psum_y = None
                with tc.tile_pool(name="ps_f", bufs=1, space="PSUM") as ps_f:
                    for j in range(hc):
                        tt_ps = ps_f.tile([P, 2], F32, tag="tp", bufs=2)
                        nc.tensor.transpose(tt_ps[:], tokc[:, j * P:(j + 1) * P],
                                            ident[:2, :2])
                        nc.vector.tensor_copy(tokT[:, j, :], tt_ps[:])

                    # G^T for the final combine (only needs gv_all; do it early
                    # so it is off the post-AllReduce critical path)
                    gt_all = sb.tile([TOPK, nt, P], F32)
                    for t in range(nt):
                        gt_ps = ps_f.tile([TOPK, P], F32, tag="tp", bufs=2)
                        nc.tensor.transpose(gt_ps[:], gv_all[:, t, :], ident[:])
                        nc.vector.tensor_copy(gt_all[:, t, :], gt_ps[:])

                    # ================= phase F: expert FFN =================
                    # psum_y col-tiled: h-bank q at partitions [32q, 32q+2)
                    psum_y = ps_f.tile([P, 512], F32)
                    w2v = W2c.ap().rearrange("(g q p) f -> p g q f", q=4, p=P)
                    fc = f // P          # 64 f-chunks of 128
                    fcq = fc // 4        # 16 f-chunks per col group
                    midg = sb.tile([P, fq], F32)       # gelu(mid), col-tiled
                    midT = sb.tile([P, fc, 2], F32)    # mid^T chunks (lhsT for W2)

                    def ffn_pass(_):
                        # ---- W1: h-major contiguous slabs; mid col-tiled:
                        # col group q holds f in [q*fq, (q+1)*fq) ----
                        mid_ps = ps_f.tile([P, fq], F32, tag="mid")
                        for j in range(hc):
                            if j < len(w_pre):
                                w1s = w_pre[j]
                            else:
                                w1s = wpool.tile([P, f], F32, tag="w")
                                nc.scalar.dma_start(w1s[:], w1v[:, j, :])
                            for q in range(4):
                                for nb in range(fq // 512):
                                    sl = slice(nb * 512, (nb + 1) * 512)
                                    nc.tensor.matmul(
                                        mid_ps[32 * q:32 * q + 2, sl],
                                        tokT[:, j, :],
                                        w1s[:, q * fq + nb * 512:
                                            q * fq + (nb + 1) * 512],
                                        start=(j == 0), stop=(j == hc - 1),
                                        tile_position=(0, 32 * q))
                        # bias + exact gelu on the whole mid at once
                        nc.vector.tensor_add(midg[:], mid_ps[:], b1t[:])
                        nc.scalar.activation(midg[:], midg[:], ACT_GELU)
                        # transpose mid chunks: global f-chunk k = q*fcq + m
                        for q in range(4):
                            for m in range(fcq):
                                mt_ps = ps_f.tile([P, 2], F32, tag="tp", bufs=2)
                                nc.tensor.transpose(
                                    mt_ps[:], midg[32 * q:32 * q + 2,
                                                   m * P:(m + 1) * P],
                                    ident[32 * q:32 * q + 2, 32 * q:32 * q + 2],
                                    tile_position=(32 * q, 0))
                                nc.vector.tensor_copy(midT[:, q * fcq + m, :],
                                                      mt_ps[:])

                        # ---- W2: contiguous row slabs, psum_y col-tiled ----
                        for g in range(ng):
                            w2s = wpool.tile([P, 4, h], F32, tag="w")
                            nc.scalar.dma_start(w2s[:], w2v[:, g, :, :])
                            for ft in range(4):
                                for q in range(hb):
                                    nc.tensor.matmul(
                                        psum_y[32 * q:32 * q + 2, :],
                                        midT[:, g * 4 + ft, :],
                                        w2s[:, ft, q * 512:(q + 1) * 512],
                                        start=(g == 0 and ft == 0),
                                        stop=(g == ng - 1 and ft == 3),
                                        tile_position=(0, 32 * q))

                    if phase_stop >= 3:
                        if ffn_reps > 1:
                            with tc.For_i(0, ffn_reps, 1):
                                ffn_pass(0)
                        else:
                            ffn_pass(0)

                    if phase_stop < 3:
                        return
                    eo_sb = sb.tile([TOPK, h], F32)
                    for q in range(hb):
                        nc.vector.tensor_copy(eo_sb[:, q * 512:(q + 1) * 512],
                                              psum_y[32 * q:32 * q + 2, :])
                    nc.vector.tensor_add(eo_sb[:], eo_sb[:], b2_sb[:])

                if phase_stop < 4:
                    return
                # ============ AllReduce: A = sum_e Eo[e] ============
                cc2_in = dram.tile([TOPK, h], F32)
                cc2_out = dram.tile([TOPK, h], F32,
                                    addr_space="Shared" if collectives else "Local")
                nc.sync.dma_start(cc2_in[:], eo_sb[:])
                if collectives:
                    nc.gpsimd.collective_compute(
                        "AllReduce", ALU.add, replica_groups=groups,
                        ins=[cc2_in.opt()], outs=[cc2_out.opt()])
                else:
                    nc.sync.dma_start(cc2_out[:], cc2_in[:])
                a_sb = sb.tile([TOPK, h], F32)
                nc.sync.dma_start(a_sb[:], cc2_out[:])

                if phase_stop < 5:
                    return
                # ================= phase C: y = G @ A =================
                with tc.tile_pool(name="ps_c", bufs=1, space="PSUM") as ps_c:
                    for t in range(nt):
                        y_sb = ypool.tile([P, h], F32, tag="y")
                        for b in range(hb):
                            yt_ps = ps_c.tile([P, 512], F32, tag="yt", bufs=2)
                            nc.tensor.matmul(yt_ps[:], gt_all[:, t, :],
                                             a_sb[:, b * 512:(b + 1) * 512],
                                             start=True, stop=True)
                            nc.vector.tensor_copy(y_sb[:, b * 512:(b + 1) * 512],
                                                  yt_ps[:])
                        nc.sync.dma_start(yo[t * P:(t + 1) * P, :], y_sb[:])


            for krep in range(kernel_reps):
                trace_once(krep)

    nc.compile()
    return nc


_NC_CACHE = {}


def _get_nc():
    if "nc" not in _NC_CACHE:
        _NC_CACHE["nc"] = build_expert_ffn()
    return _NC_CACHE["nc"]


def kernel(x, Wr, br, W1, b1, W2, b2):
    x = np.ascontiguousarray(np.asarray(x, dtype=np.float32))
    Wr = np.ascontiguousarray(np.asarray(Wr, dtype=np.float32))
    br = np.ascontiguousarray(np.asarray(br, dtype=np.float32))
    W1 = np.ascontiguousarray(np.asarray(W1, dtype=np.float32))
    b1 = np.ascontiguousarray(np.asarray(b1, dtype=np.float32))
    W2 = np.ascontiguousarray(np.asarray(W2, dtype=np.float32))
    b2 = np.ascontiguousarray(np.asarray(b2, dtype=np.float32))

    nc = _get_nc()
    tn = N // NCORES
    x2 = x.reshape(N, H)
    in_maps = []
    for c in range(NCORES):
        in_maps.append({
            "xs": x2[c * tn:(c + 1) * tn],
            "Wr": Wr,
            "brb": br.reshape(1, E),
            "W1c": W1[c],
            "b1c": b1[c].reshape(1, F),
            "W2c": W2[c],
            "b2c": b2[c].reshape(1, H),
        })
    res = run_bass_kernel_spmd(nc, in_maps, list(range(NCORES)), trace=False)
    y = np.concatenate([res.results[c]["y"] for c in range(NCORES)], axis=0)
    return y.reshape(B, S, H)

